# revision 19
# baseline (speedup 1.0000x reference)
"""Trainium2 Bass kernel for a basis-customized linear layer.

Reference computation (B=1024, IN=OUT=512, EMB=64, KQ=64, NB=3, VOCAB=100):
    embs = concat(emb_author[idx_author], emb_citation[idx_citation])  # [B, 128]
    h    = tanh(embs @ W1.T + b1)                                      # [B, 64]
    coef = softmax(h @ W2.T)                                           # [B, 3]
    w    = (coef @ W3.T + b3).reshape(B, IN, OUT)
    out  = einsum('bi,bio->bo', x, w)                                  # [B, 512]

Rewrites:
  (1) w[b] = sum_j coef[b,j]*W3j + b3r and softmax coefs sum to 1, so
      out = sum_j coef[:,j] * (x @ (W3j + b3r)) -- three shared [512,512]
      matmuls instead of the 1GB per-sample weight.
  (2) Y_j[m] = x_m @ W_j accumulate over k into per-(m,j) PSUM banks; the
      per-sample combine sum_j coef[b,j]*Y_j[b,o] has b on PARTITIONS, so
      coef enters as a per-partition [128,1] scalar: one ACT scale-copy +
      two fused DVE scalar_tensor_tensor ((Y*c) + acc) ops per m-tile,
      pipelined behind the j-th matmul group.  Nothing gates the main
      matmuls except the x/W DMAs.
  (3) the embedding gather is a one-hot matmul (idx compared against an
      iota via a single fused K=2 matmul covering author+citation), fused
      with W1 through the host-precomputed per-vocab table
      G = emb @ W1half.T (param-only fold).
  (4) everything travels bf16 (half the HBM bytes, 2x PE rate); PSUM
      accumulation stays f32.
  (5) all small operands ride ONE DMA (sm): gather tables + W2.T|b1 on
      partitions 0..99, idx/iota rows on partitions 64-65 (PE operands
      must start at partition 0/32/64).  4 input DMAs total, split across
      the scalar/sync/gpsimd sequencers (a dma_start costs ~0.7-1.2us of
      sequencer issue time, and each DMA dependency pays ~1.5us of
      completion-semaphore latency).

Sharding over 8 cores: batch 4-way x out-column 2-way.
"""

import numpy as np
import ml_dtypes

import concourse.bass as bass
import concourse.tile as tile
from concourse import bacc, mybir
from concourse.bass_utils import run_bass_kernel_spmd

# Problem dims (hardcoded per contract)
B, IN, OUT = 1024, 512, 512
EMB, KQ, NB, VOCAB = 64, 64, 3, 100
P_B, Q_O = 4, 2            # batch shards x out-col shards = 8 cores
BS = B // P_B              # 256 batch rows per core
OW = OUT // Q_O            # 256 out cols per core
KT = IN // 128             # 4 contraction tiles
MT = BS // 128             # 2 batch tiles per core

F32 = mybir.dt.float32
F32R = mybir.dt.float32r
BF16 = mybir.dt.bfloat16

# smS (tiny, lands first; gates the one-hot):  [2, 612] bf16
#   cols [0:256) idx_author | ones ; [256:512) idx_citation | ones
#   cols [512:612) ones | -iota(VOCAB)
SMS = 2 * BS + VOCAB
# smT (gather tables): [128, 132] bf16
#   cols [0:64) gat rows 0:100, [64:128) gct, [128:132) W2.T | b1 rows 0:64
SMT = 2 * KQ + NB + 1

LAST_RESULT = None         # BassKernelResults of the most recent run (for test.py)

_NC_CACHE = None


def _ensure_ntff_hook_module():
    """bass_utils imports antenv.axon_hooks when BASS_TRACE is set; the module
    is absent on this image. Provide a no-op shim so tracing degrades
    gracefully instead of crashing."""
    import sys, types
    if "antenv.axon_hooks" in sys.modules:
        return
    try:
        import antenv
        import antenv.axon_hooks  # noqa: F401
    except ImportError:
        mod = types.ModuleType("antenv.axon_hooks")
        state = {"hook": None}
        mod.set_axon_ntff_profile_hook = lambda h: state.__setitem__("hook", h)
        mod.get_axon_ntff_profile_hook = lambda: state["hook"]
        sys.modules["antenv.axon_hooks"] = mod
        try:
            antenv.axon_hooks = mod
        except Exception:
            pass


def _build_nc():
    nc = bacc.Bacc("TRN2", target_bir_lowering=False, debug=False,
                   num_devices=P_B * Q_O)

    xt = nc.dram_tensor("xt", [128, KT * BS], BF16, kind="ExternalInput")
    wc = nc.dram_tensor("wc", [128, KT * NB * OW], BF16, kind="ExternalInput")
    sms = nc.dram_tensor("sms", [2, SMS], BF16, kind="ExternalInput")
    smt = nc.dram_tensor("smt", [128, SMT], BF16, kind="ExternalInput")
    out = nc.dram_tensor("out", [128, MT * OW], BF16, kind="ExternalOutput")

    with tile.TileContext(nc) as tc:
        with (
            tc.tile_pool(name="consts", bufs=1) as consts,
            tc.tile_pool(name="ps", bufs=1, space="PSUM") as ps,
        ):
            # ---- input DMAs split across the three DMA-capable seqs ----
            # gating order: sms (one-hot) -> smt/xt/wck0 -> wck1 -> wck23
            sms_sb = consts.tile([2, SMS], BF16)
            nc.sync.dma_start(out=sms_sb, in_=sms[:, :])
            smt_sb = consts.tile([128, SMT], BF16)
            nc.scalar.dma_start(out=smt_sb, in_=smt[:, :])
            xt_sb = consts.tile([128, KT, BS], BF16)
            nc.scalar.dma_start(
                out=xt_sb, in_=xt[:, :].rearrange("p (k n) -> p k n", k=KT))
            wc_sb = consts.tile([128, KT * NB * OW], BF16)
            kc = NB * OW
            nc.sync.dma_start(out=wc_sb[:, 0:kc], in_=wc[:, 0:kc])
            nc.sync.dma_start(out=wc_sb[:, kc:2 * kc], in_=wc[:, kc:2 * kc])
            nc.scalar.dma_start(out=wc_sb[:, 2 * kc:], in_=wc[:, 2 * kc:])

            gat_sb = smt_sb[0:VOCAB, 0:KQ]
            gct_sb = smt_sb[0:VOCAB, KQ:2 * KQ]
            w2r_sb = smt_sb[0:KQ, 2 * KQ:SMT]          # W2.T | b1 pad col
            idx_sb = sms_sb[:, 0:2 * BS]               # author | citation
            bw_sb = sms_sb[:, 2 * BS:SMS]

            # ---- PE stream (in-order): interleave stage-A with main ----
            # warm-up: the HAM clock-gate needs ~3.4us of continuous PE
            # activity before matmuls run at full rate.  Burn the DMA-wait
            # window with dummy matmuls on memset-zero data; each one
            # start=True-clears the bank the real one-hot then overwrites.
            zz_sb = consts.tile([2, 512], BF16)
            nc.vector.memset(zz_sb, 0)
            oh_ps = ps.tile([VOCAB, 2 * BS], F32, tag="oh", bufs=1,
                            padded_shape=[128, 512])
            pre_ps = ps.tile([KQ, BS], F32, tag="pre", bufs=1,
                             padded_shape=[128, 512])
            for _ in range(17):
                nc.tensor.matmul(oh_ps[:, 0:128], lhsT=zz_sb[:, 0:VOCAB],
                                 rhs=zz_sb[:, 0:128], start=True, stop=True)

            # fused one-hot for both embeddings: psum[v, (a|c)b]
            nc.tensor.matmul(oh_ps, lhsT=bw_sb, rhs=idx_sb, start=True, stop=True)
            oh_sb = consts.tile([VOCAB, 2 * BS], BF16)
            nc.vector.tensor_scalar(
                out=oh_sb[:, 0:BS], in0=oh_ps[:, 0:BS], scalar1=0.0,
                scalar2=None, op0=mybir.AluOpType.is_equal,
            )
            nc.vector.tensor_scalar(
                out=oh_sb[:, BS:2 * BS], in0=oh_ps[:, BS:2 * BS], scalar1=0.0,
                scalar2=None, op0=mybir.AluOpType.is_equal,
            )

            # keep the PE busy until the k0 chunk's semaphore fires
            for _ in range(7):
                nc.tensor.matmul(pre_ps[:, 0:128], lhsT=zz_sb[:, 0:KQ],
                                 rhs=zz_sb[:, 0:128], start=True, stop=True)

            y_ps = [[ps.tile([128, OW], F32, tag="y", bufs=2 * NB,
                             name=f"y{m}_{j}", padded_shape=[128, 512])
                     for j in range(NB)] for m in range(MT)]

            def mm(m, j, k):
                cs = (k * NB + j) * OW
                nc.tensor.matmul(
                    y_ps[m][j],
                    lhsT=xt_sb[:, k, m * 128:(m + 1) * 128],
                    rhs=wc_sb[:, cs:cs + OW],
                    start=(k == 0), stop=(k == KT - 1),
                )

            # main matmuls k-major: 6 MMs per k-tile chase the wc chunks;
            # stage-A MMs slot into the stream right when their deps land
            for m in range(MT):
                for j in range(NB):
                    mm(m, j, 0)

            # fused gather + W1: preact.T [KQ, BS]
            nc.tensor.matmul(pre_ps, lhsT=gat_sb, rhs=oh_sb[:, 0:BS],
                             start=True, stop=False)
            nc.tensor.matmul(pre_ps, lhsT=gct_sb, rhs=oh_sb[:, BS:2 * BS],
                             start=False, stop=True)

            for m in range(MT):
                for j in range(NB):
                    mm(m, j, 1)

            # b1 routed through ACT so Tanh's bias dep is same-engine
            b1_sb = consts.tile([KQ, 1], F32)
            nc.scalar.copy(out=b1_sb, in_=smt_sb[0:KQ, SMT - 1:SMT])
            ht_sb = consts.tile([KQ, BS], BF16)
            nc.scalar.activation(
                out=ht_sb, in_=pre_ps, func=mybir.ActivationFunctionType.Tanh,
                bias=b1_sb, scale=1.0,
            )

            # per-m logits -> unnormalized e -> coef = e / sum(e)
            # lg0 reuses the one-hot's PSUM bank, lg1 the preact's
            coef = []
            for m in range(MT):
                lg_ps = ps.tile([128, NB + 1], F32, tag="oh" if m == 0 else "pre",
                                bufs=1, padded_shape=[128, 512])
                nc.tensor.matmul(
                    lg_ps, lhsT=ht_sb[:, m * 128:(m + 1) * 128], rhs=w2r_sb,
                    start=True, stop=True,
                )
                e_sb = consts.tile([128, NB], F32, name=f"e{m}")
                nc.scalar.activation(
                    out=e_sb, in_=lg_ps[:, 0:NB],
                    func=mybir.ActivationFunctionType.Exp,
                )
                s_sb = consts.tile([128, 1], F32, name=f"s{m}")
                nc.vector.reduce_sum(out=s_sb, in_=e_sb, axis=mybir.AxisListType.X)
                r_sb = consts.tile([128, 1], F32, name=f"r{m}")
                nc.vector.reciprocal(out=r_sb, in_=s_sb)
                cf = consts.tile([128, NB], F32, name=f"coef{m}")
                nc.vector.tensor_scalar(
                    out=cf, in0=e_sb, scalar1=r_sb, scalar2=None,
                    op0=mybir.AluOpType.mult,
                )
                coef.append(cf)

            for m in range(MT):
                for j in range(NB):
                    mm(m, j, 2)
            # last k-tile m-interleaved so both combines pipeline behind it
            for j in range(NB):
                for m in range(MT):
                    mm(m, j, 3)

            # ---- combine: out[m] = sum_j coef[:,j] * Y_j[m] ----
            a_sb = [consts.tile([128, OW], F32, name=f"a{m}") for m in range(MT)]
            b_sb = [consts.tile([128, OW], F32, name=f"b{m}") for m in range(MT)]
            out_sb = [consts.tile([128, OW], BF16, name=f"o{m}") for m in range(MT)]
            for m in range(MT):
                nc.scalar.activation(
                    out=a_sb[m], in_=y_ps[m][0],
                    func=mybir.ActivationFunctionType.Copy,
                    scale=coef[m][:, 0:1],
                )
            for m in range(MT):
                nc.vector.scalar_tensor_tensor(
                    out=b_sb[m], in0=y_ps[m][1], scalar=coef[m][:, 1:2],
                    in1=a_sb[m], op0=mybir.AluOpType.mult, op1=mybir.AluOpType.add,
                )
            for m in range(MT):
                nc.vector.scalar_tensor_tensor(
                    out=out_sb[m], in0=y_ps[m][2], scalar=coef[m][:, 2:3],
                    in1=b_sb[m], op0=mybir.AluOpType.mult, op1=mybir.AluOpType.add,
                )

            nc.scalar.dma_start(out=out[:, 0:OW], in_=out_sb[0])
            nc.sync.dma_start(out=out[:, OW:2 * OW], in_=out_sb[1])

    nc.compile()
    return nc


def _get_nc():
    global _NC_CACHE
    if _NC_CACHE is None:
        _NC_CACHE = _build_nc()
    return _NC_CACHE


def _make_in_maps(x, idx_author, idx_citation, emb_author, emb_citation,
                  W1, b1, W2, W3, b3):
    f = np.float32
    bf = ml_dtypes.bfloat16
    x = np.asarray(x, dtype=f)
    W3r = np.asarray(W3, dtype=f).reshape(IN, OUT, NB)
    b3r = np.asarray(b3, dtype=f).reshape(IN, OUT)
    W1 = np.asarray(W1, dtype=f)

    # param-only folds: per-vocab gather tables G = emb @ W1half.T [VOCAB, KQ]
    smt = np.zeros((128, SMT), bf)
    smt[:VOCAB, :KQ] = (np.asarray(emb_author, dtype=f) @ W1[:, :EMB].T).astype(bf)
    smt[:VOCAB, KQ:2 * KQ] = (np.asarray(emb_citation, dtype=f) @ W1[:, EMB:].T).astype(bf)
    smt[:KQ, 2 * KQ:2 * KQ + NB] = np.asarray(W2, dtype=f).T.astype(bf)
    smt[:KQ, SMT - 1] = np.asarray(b1, dtype=f).astype(bf)
    sms = np.zeros((2, SMS), bf)
    sms[0, 2 * BS:] = 1
    sms[1, 2 * BS:] = (-np.arange(VOCAB, dtype=f)).astype(bf)
    sms[1, 0:2 * BS] = 1

    # per out-shard weight blocks, bias folded in: [128, k, j, ow] bf16
    wc_blocks = []
    for oj in range(Q_O):
        cols = slice(oj * OW, (oj + 1) * OW)
        blk = W3r[:, cols, :] + b3r[:, cols, None]       # [IN, OW, NB]
        blk = blk.reshape(KT, 128, OW, NB).transpose(1, 0, 3, 2)
        wc_blocks.append(np.ascontiguousarray(
            blk.reshape(128, KT * NB * OW).astype(bf)))

    # x.T per batch shard, k packed: [128, KT*BS] bf16
    xt_shards = []
    for bi in range(P_B):
        xs = x[bi * BS:(bi + 1) * BS, :].T               # [IN, BS]
        xs = xs.reshape(KT, 128, BS).transpose(1, 0, 2)
        xt_shards.append(np.ascontiguousarray(
            xs.reshape(128, KT * BS).astype(bf)))

    ia = np.asarray(idx_author).astype(bf)
    ic = np.asarray(idx_citation).astype(bf)

    in_maps = []
    for c in range(P_B * Q_O):
        bi, oj = c // Q_O, c % Q_O  # 4 batch shards x 2 out shards
        rows = slice(bi * BS, (bi + 1) * BS)
        smsc = sms.copy()
        smsc[0, 0:BS] = ia[rows]
        smsc[0, BS:2 * BS] = ic[rows]
        in_maps.append({
            "xt": xt_shards[bi],
            "wc": wc_blocks[oj],
            "sms": smsc,
            "smt": smt,
        })
    return in_maps


def kernel(x, idx_author, idx_citation, emb_author, emb_citation,
           W1, b1, W2, W3, b3):
    global LAST_RESULT
    _ensure_ntff_hook_module()
    nc = _get_nc()
    in_maps = _make_in_maps(x, idx_author, idx_citation, emb_author,
                            emb_citation, W1, b1, W2, W3, b3)
    res = run_bass_kernel_spmd(nc, in_maps, core_ids=list(range(P_B * Q_O)))
    LAST_RESULT = res
    out = np.empty((B, OUT), dtype=np.float32)
    for c in range(P_B * Q_O):
        bi, oj = c // Q_O, c % Q_O
        blk = np.asarray(res.results[c]["out"], dtype=np.float32)
        blk = blk.reshape(128, MT, OW).transpose(1, 0, 2)
        out[bi * BS:(bi + 1) * BS, oj * OW:(oj + 1) * OW] = \
            blk.reshape(BS, OW)
    return out
